# revision 33
# baseline (speedup 1.0000x reference)
"""Trainium2 Bass kernel for nn_Attention_42288247996512 (sparse causal cross-attention).

reference:
  q = x @ Wq.T; k = cross @ Wk.T; v = x @ Wv.T
  logits = q @ k.T  (causal mask; padding mask m_q*m_k + eye > 0)
  out = softmax(logits / sqrt(128)) @ v

Sharding: 8 cores = 4 batches x 2 query-strips (SPMD), 1024 query rows per
core.  The strip's column order is host-chosen: [unmasked queries | masked
queries], with unmasked queries split between the batch's two strips by
even/odd compacted index so both strips have near-identical causal
structure.

Structural optimizations vs a vanilla flash-style kernel:

1) Reassociation:  attn @ (x @ Wv.T) == (attn @ x) @ Wv.T.  Each core owns
   1024 query rows but would need all 2048 key rows of v, so projecting
   t = attn@x instead of v halves that matmul.

2) Key compaction: ~half the keys are padding-masked (exp == 0 columns).
   Keys are host-compacted to the active ones, shrinking
   kT/logits/exp/transpose/AX nearly 2x.

3) Query compaction: masked queries have softmax == delta(diagonal), i.e.
   out[q] = v[q].  Attention (qT/logits/exp/AX) runs only on the ~unmasked
   column prefix; x.T is DMA'd into the t buffer so the masked columns pass
   through t @ Wv.T and produce v[q] directly.  Attention results are merged
   over the pass-through with predicated copies (mask = column-is-unmasked).
   The pure-pass-through tail columns' t @ Wv.T segment has no attention
   dependency, providing early PE work that overlaps the input DMA stream.

The kernel program is JIT-specialized on the mask's *structure* (block
counts / chunk widths / which chunks need additive masks, maxed over the 8
cores so one SPMD program serves all).  Structure parameters are recomputed
from the input mask on every call (and cached), so the kernel stays correct
for any input.  All streamed operands are bf16; PSUM accumulation is f32.
Host does layout packs/bf16 casts, gathers, additive-mask building, and the
final denominator divide + scatter (as in the baseline kernel).
"""
import math
import threading

import ml_dtypes
import numpy as np

B, S, D, DA = 4, 2048, 1024, 128
P = 128
NCORES = 8
BIG = 32768.0  # power of two: exactly representable in bf16
NQ = 1024      # query rows per core strip
KC = D // P    # 8 contraction chunks of 128

_BUILD_LOCK = threading.Lock()
_CACHE: dict = {}


def _strip_queries(mb, p):
    """Column order for strip p of a batch: unmasked (even/odd compacted
    interleave) first, then masked, filled so each strip has exactly NQ
    rows and the two strips partition the batch's 2048 rows."""
    uq = np.nonzero(mb)[0]
    mq = np.nonzero(~mb)[0]
    u = uq[p::2]
    w0 = NQ - len(uq[0::2])
    mtake = mq[:w0] if p == 0 else mq[w0:]
    return u, mtake


def _derive_params(mask_f):
    """SPMD kernel structure (maxed over all 8 cores) from the mask.

    Returns (nkb, nqb, kb, chunks, masked):
      nkb:   compacted key blocks (128 each)
      nqb:   attention query blocks per strip (128 each)
      kb:    per-pair key-block count (AX contraction length)
      chunks: per-pair tuple of logits chunk widths (<=512, multiples of 128)
      masked: per-pair tuple of bools - does chunk j need an additive mask
    """
    nkb = 0
    nqb = 0
    stats = []  # per (b, p): (ck, uq, nk)
    for b in range(B):
        m = mask_f[b] > 0
        ck = np.cumsum(m)
        nk = int(ck[-1])
        nkb = max(nkb, (nk + P - 1) // P)
        for p in range(2):
            uq, _ = _strip_queries(m, p)
            nqb = max(nqb, (len(uq) + P - 1) // P)
            stats.append((ck, uq, nk))
    npair = (nqb + 1) // 2
    kb = [0] * npair
    for ck, uq, nk in stats:
        for pr in range(npair):
            qs = uq[pr * 256:min((pr + 1) * 256, len(uq))]
            if len(qs) == 0:
                continue
            bmax = int(ck[qs].max())
            kb[pr] = max(kb[pr], (bmax + P - 1) // P)
    chunks = []
    for pr in range(npair):
        w = kb[pr] * P
        ch = []
        while w > 0:
            ch.append(min(512, w))
            w -= min(512, w)
        chunks.append(tuple(ch))
    masked = [[False] * len(chunks[pr]) for pr in range(npair)]
    for ck, uq, nk in stats:
        for pr in range(npair):
            qs = uq[pr * 256:min((pr + 1) * 256, len(uq))]
            bmin = int(ck[qs].min()) if len(qs) else 0
            base = 0
            for j, w in enumerate(chunks[pr]):
                if base + w > bmin or base + w > nk:
                    masked[pr][j] = True
                base += w
    return (nkb, nqb, tuple(kb), tuple(chunks),
            tuple(tuple(mj) for mj in masked))


def _build(params):
    from contextlib import ExitStack

    import concourse.bass as bass
    import concourse.mybir as mybir
    import concourse.tile as tile
    from concourse import bacc
    from concourse.masks import make_identity

    nkb, nqb, KB, CHUNKS, MASKED = params
    NK = max(nkb, 1) * P             # padded compacted key width
    NPAIR = (nqb + 1) // 2           # attention column pairs (<=256 each)
    QCOL = nqb * P                   # attention columns
    nt = sum(2 * sum(mj) for mj in MASKED)
    NT = max(nt, 1)

    def pblocks(pr):
        return 1 if (pr == NPAIR - 1 and nqb % 2 == 1) else 2

    dt = mybir.dt
    f32 = dt.float32
    bf16 = dt.bfloat16
    u8 = dt.uint8
    AF = mybir.ActivationFunctionType
    ALU = mybir.AluOpType

    nc = bacc.Bacc("TRN2", target_bir_lowering=False, debug=False)

    xkc = nc.dram_tensor("xkc", [NK, D], bf16, kind="ExternalInput").ap()
    cTc = nc.dram_tensor("cTc", [D, NK], bf16, kind="ExternalInput").ap()
    xqT = nc.dram_tensor("xqT", [D, NQ], bf16, kind="ExternalInput").ap()
    wqp = nc.dram_tensor("wqp", [P, KC, DA], bf16, kind="ExternalInput").ap()
    wkp = nc.dram_tensor("wkp", [P, KC, DA], bf16, kind="ExternalInput").ap()
    wvp = nc.dram_tensor("wvp", [P, KC, D], bf16, kind="ExternalInput").ap()
    dm2 = nc.dram_tensor("dm2", [NT, P, 512], bf16, kind="ExternalInput").ap()
    pmask = nc.dram_tensor("pmask", [P, max(NPAIR, 1), 256], u8,
                           kind="ExternalInput").ap()

    outT = nc.dram_tensor("outT", [D, NQ], bf16,
                          kind="ExternalOutput").ap()
    den = nc.dram_tensor("den", [P, max(nqb, 1)], f32,
                         kind="ExternalOutput").ap()

    xkc_r = xkc.rearrange("(kb p) d -> p kb d", p=P)
    cTc_r = cTc.rearrange("(kc p) s -> p kc s", p=P)
    xqT_r = xqT.rearrange("(kc p) q -> p kc q", p=P)
    outT_r = outT.rearrange("(dmc p) q -> p dmc q", p=P)

    dmidx = {}

    def dm_tile(pr, j, blk):
        key = (pr, j, blk)
        if key not in dmidx:
            dmidx[key] = len(dmidx)
        return dmidx[key]

    kt_chunks = []
    w = NK
    while w > 0:
        kt_chunks.append(min(512, w))
        w -= min(512, w)

    with tile.TileContext(nc) as tc, ExitStack() as ctx:
        const = ctx.enter_context(tc.tile_pool(name="const", bufs=1))
        persist = ctx.enter_context(tc.tile_pool(name="persist", bufs=1))
        stream = ctx.enter_context(tc.tile_pool(name="stream", bufs=2))
        apool = ctx.enter_context(tc.tile_pool(name="apool", bufs=4))
        epool = ctx.enter_context(tc.tile_pool(name="epool", bufs=24))

        ident_f32 = const.tile([P, P], f32, name="ident_f32")
        make_identity(nc, ident_f32)
        ident = const.tile([P, P], bf16, name="ident")
        nc.vector.tensor_copy(ident[:], ident_f32[:])

        wq_sb = const.tile([P, KC, DA], bf16, name="wq_sb")
        wk_sb = const.tile([P, KC, DA], bf16, name="wk_sb")
        wv_sb = const.tile([P, KC, D], bf16, name="wv_sb")
        dm_sb = const.tile([P, NT, 512], bf16, name="dm_sb")
        pm_sb = const.tile([P, max(NPAIR, 1), 256], u8, name="pm_sb")

        kT_sb = persist.tile([P, NK], bf16, name="kT_sb")
        qT_sb = persist.tile([P, max(QCOL, P)], bf16, name="qT_sb")
        xk_sb = persist.tile([P, max(nkb, 1), D], bf16, name="xk_sb")
        tT_sb = persist.tile([P, KC, NQ], bf16, name="tT_sb")
        den_sb = persist.tile([P, max(nqb, 1)], f32, name="den_sb")

        eTs_all = {pr: [] for pr in range(NPAIR)}
        daccs_all = {pr: [[], []] for pr in range(NPAIR)}

        psl_pool = ctx.enter_context(
            tc.tile_pool(name="psl", bufs=2, space="PSUM"))
        psT_pool = ctx.enter_context(
            tc.tile_pool(name="psT", bufs=2, space="PSUM"))
        psax_pool = ctx.enter_context(
            tc.tile_pool(name="psax", bufs=2, space="PSUM"))
        psw_pool = ctx.enter_context(
            tc.tile_pool(name="psw", bufs=2, space="PSUM"))

        # PE warmup: dependency-free matmuls keep the PE busy (and its
        # p-state at full clock) while the first input DMAs stream in.
        _wu = [0]

        def warm(n):
            for _ in range(n):
                pswu = psw_pool.tile([P, P], f32, tag="psw",
                                     name=f"pswu{_wu[0]}",
                                     padded_shape=[P, 512])
                _wu[0] += 1
                nc.tensor.matmul(pswu[:], lhsT=ident[:], rhs=ident[:],
                                 start=True, stop=True)

        warm(48)

        # ---- projections (DMA emission order == SP FIFO delivery order) ----
        def kT_chunk(j):
            if nkb == 0:
                return
            w = kt_chunks[j]
            base = sum(kt_chunks[:j])
            ctj = stream.tile([P, KC, 512], bf16, tag="ct", name=f"ct{j}",
                              bufs=2)
            nc.sync.dma_start(ctj[:, :, :w], cTc_r[:, :, base:base + w])
            ps_k = psax_pool.tile([P, 512], f32, tag="psax", name=f"ps_k{j}")
            for kc in range(KC):
                nc.tensor.matmul(
                    ps_k[:, :w],
                    lhsT=wk_sb[:, kc, :],
                    rhs=ctj[:, kc, :w],
                    start=(kc == 0), stop=(kc == KC - 1),
                )
            nc.any.tensor_copy(kT_sb[:, base:base + w], ps_k[:, :w])

        nc.sync.dma_start(wq_sb[:], wqp)
        # qT over the attention columns only; rhs is the tT fill itself
        qch = []
        w = QCOL
        while w > 0:
            qch.append(min(512, w))
            w -= min(512, w)
        ps_q = [psl_pool.tile([P, 512], f32, tag="psl", name=f"ps_q{n}")
                for n in range(len(qch))]

        def qT_fill(c0, c1):
            nc.sync.dma_start(tT_sb[:, :, c0:c1], xqT_r[:, :, c0:c1])

        def qT_mm(n):
            base = sum(qch[:n])
            for kc in range(KC):
                nc.tensor.matmul(
                    ps_q[n][:, :qch[n]],
                    lhsT=wq_sb[:, kc, :],
                    rhs=tT_sb[:, kc, base:base + qch[n]],
                    start=(kc == 0), stop=(kc == KC - 1),
                )

        if qch:
            qT_fill(0, qch[0])
            qT_mm(0)
        for n in range(1, len(qch)):
            base = sum(qch[:n])
            qT_fill(base, base + qch[n])
        nc.sync.dma_start(wk_sb[:], wkp)
        kT_chunk(0)
        for n in range(1, len(qch)):
            qT_mm(n)
        for n in range(len(qch)):
            base = sum(qch[:n])
            nc.any.tensor_copy(qT_sb[:, base:base + qch[n]],
                               ps_q[n][:, :qch[n]])
        NTA = min(6, NT)
        nc.sync.dma_start(dm_sb[:, 0:NTA, :],
                          dm2.rearrange("t p w -> p t w")[:, 0:NTA, :])
        nc.sync.dma_start(pm_sb[:], pmask)

        # ---- attention stages ----
        def stage_a_chunk(pr, j, mid_hook=None):
            w = CHUNKS[pr][j]
            base = sum(CHUNKS[pr][:j])
            nks = (w + P - 1) // P
            nbl = pblocks(pr)
            psTs = [psT_pool.tile([P, nbl * P], bf16, tag="psT",
                                  name=f"psT{pr}_{j}_{ks}",
                                  padded_shape=[P, 512])
                    for ks in range(nks)]
            es = []
            for blk in range(nbl):
                slot = pr * 2 + blk
                psl = psl_pool.tile([P, 512], f32, tag="psl",
                                    name=f"psl{slot}_{j}")
                nc.tensor.matmul(
                    psl[:, :w],
                    lhsT=qT_sb[:, slot * P:(slot + 1) * P],
                    rhs=kT_sb[:, base:base + w],
                    start=True, stop=True,
                )
                dac = apool.tile([P, 1], f32, tag="dac",
                                 name=f"dac{slot}_{j}", bufs=12)
                e = apool.tile([P, 512], bf16, tag="e", name=f"e{slot}_{j}")
                if MASKED[pr][j]:
                    sbl = apool.tile([P, 512], f32, tag="sbl",
                                     name=f"sbl{slot}_{j}")
                    nc.vector.tensor_tensor(
                        out=sbl[:, :w], in0=psl[:, :w],
                        in1=dm_sb[:, dm_tile(pr, j, blk), :w], op=ALU.add)
                    src = sbl
                else:
                    src = psl
                if w > 256:
                    dac2 = apool.tile([P, 1], f32, tag="dac",
                                      name=f"dac2_{slot}_{j}", bufs=12)
                    nc.scalar.activation(
                        e[:, :256], src[:, :256], AF.Exp,
                        bias=0.0, scale=1.0, accum_out=dac[:])
                    nc.scalar.activation(
                        e[:, 256:w], src[:, 256:w], AF.Exp,
                        bias=0.0, scale=1.0, accum_out=dac2[:])
                    daccs_all[pr][blk].append(dac2)
                else:
                    nc.scalar.activation(
                        e[:, :w], src[:, :w], AF.Exp,
                        bias=0.0, scale=1.0, accum_out=dac[:])
                daccs_all[pr][blk].append(dac)
                es.append(e)
            if mid_hook is not None:
                mid_hook()
            for blk in range(nbl):
                for ks in range(nks):
                    nc.tensor.transpose(
                        psTs[ks][:, blk * P:(blk + 1) * P],
                        es[blk][:, ks * P:(ks + 1) * P],
                        ident[:],
                    )
            for ks in range(nks):
                eT = epool.tile([P, nbl * P], bf16, tag="eT",
                                name=f"eT{pr}_{j}_{ks}")
                nc.any.tensor_copy(eT[:], psTs[ks][:])
                eTs_all[pr].append(eT)

        def stage_den(pr):
            for blk in range(pblocks(pr)):
                slot = pr * 2 + blk
                dl = daccs_all[pr][blk]
                dst = den_sb[:, slot:slot + 1]
                if len(dl) == 1:
                    nc.any.tensor_copy(dst, dl[0][:])
                else:
                    nc.vector.tensor_tensor(
                        out=dst, in0=dl[0][:], in1=dl[1][:], op=ALU.add)
                    for d in dl[2:]:
                        nc.vector.tensor_tensor(
                            out=dst, in0=dst, in1=d[:], op=ALU.add)

        def stage_ax(pr, mid_hook=None):
            stage_den(pr)
            eTs = eTs_all[pr]
            wq_ = pblocks(pr) * P
            for xdc in range(KC):
                psax = psax_pool.tile([P, wq_], f32, tag="psax",
                                      name=f"psax{pr}_{xdc}",
                                      padded_shape=[P, 512])
                for kb in range(KB[pr]):
                    nc.tensor.matmul(
                        psax[:],
                        lhsT=xk_sb[:, kb, xdc * P:(xdc + 1) * P],
                        rhs=eTs[kb][:],
                        start=(kb == 0), stop=(kb == KB[pr] - 1),
                    )
                nc.vector.copy_predicated(
                    tT_sb[:, xdc, pr * 256:pr * 256 + wq_],
                    pm_sb[:, pr, :wq_], psax[:])
                if mid_hook is not None and xdc == 3:
                    mid_hook()

        _osbs = {}

        def twv_seg(c0, c1, dm_half, osb_key, fine=False, den_dma=False):
            # out.T[dm, c0:c1] = Wv.T.T @ t.T columns [c0, c1)
            wseg = c1 - c0
            osb = _osbs.setdefault(
                osb_key, apool.tile([P, KC, 512], bf16, tag="osb",
                                    name=f"osb{osb_key}", bufs=2))
            del den_dma
            for dmc in range(4 * dm_half, 4 * dm_half + 4):
                psw = psw_pool.tile([P, 512], f32, tag="psw",
                                    name=f"psw{osb_key}_{dmc}")
                for xdc in range(KC):
                    nc.tensor.matmul(
                        psw[:, :wseg],
                        lhsT=wv_sb[:, xdc, dmc * P:(dmc + 1) * P],
                        rhs=tT_sb[:, xdc, c0:c1],
                        start=(xdc == 0), stop=(xdc == KC - 1),
                    )
                if dmc % 2 == 0:
                    nc.vector.tensor_copy(osb[:, dmc, :wseg],
                                          psw[:, :wseg])
                else:
                    nc.scalar.copy(osb[:, dmc, :wseg], psw[:, :wseg])
                if fine:
                    eng = nc.scalar if dmc % 2 == 0 else nc.sync
                    eng.dma_start(
                        outT_r[:, dmc:dmc + 1, c0:c1],
                        osb[:, dmc:dmc + 1, :wseg])
            if not fine:
                nc.scalar.dma_start(
                    outT_r[:, 4 * dm_half:4 * dm_half + 4, c0:c1],
                    osb[:, 4 * dm_half:4 * dm_half + 4, :wseg])

        def A(pr, j, mid_hook=None):
            if pr < NPAIR and j < len(CHUNKS[pr]) and CHUNKS[pr][j] > 0:
                stage_a_chunk(pr, j, mid_hook=mid_hook)
            elif mid_hook is not None:
                mid_hook()

        def AX(pr, mid_hook=None):
            if pr < NPAIR and KB[pr] > 0:
                stage_ax(pr, mid_hook=mid_hook)
            elif mid_hook is not None:
                mid_hook()

        # ---- fused schedule (PE emission order tuned to DMA arrivals) ----
        tails = []
        c = QCOL
        while c < NQ:
            c2 = min(c + 512, NQ)
            tails.append((c, c2))
            c = c2

        nch = [len(CHUNKS[pr]) for pr in range(NPAIR)]
        kpref = sorted(set(
            [min(KB[pr], nkb) for pr in range(NPAIR)] + [nkb]))
        kpref = [k for k in kpref if k > 0]

        # pull an unmasked chunk of a later pair into AX(0)'s copy stalls
        hooked = []

        def ax0_hook():
            for pr in range(2, NPAIR):
                if nch[pr] > 0 and not MASKED[pr][0]:
                    hooked.append((pr, 0))
                    A(pr, 0)
                    return

        A(0, 0, mid_hook=lambda: kT_chunk(1) if len(kt_chunks) > 1 else None)
        if NPAIR > 1:
            A(1, 0, mid_hook=lambda: [kT_chunk(j) for j in
                                      range(2, len(kt_chunks))])
            A(1, 1)
        if kpref:
            nc.sync.dma_start(xk_sb[:, 0:kpref[0], :],
                              xkc_r[:, 0:kpref[0], :])
        # pure pass-through tail fill (no attention dependency)
        if QCOL < NQ:
            nc.sync.dma_start(tT_sb[:, :, QCOL:NQ], xqT_r[:, :, QCOL:NQ])
        AX(0, mid_hook=ax0_hook)
        nc.sync.dma_start(wv_sb[:, :, 0:512], wvp[:, :, 0:512])
        # early pure-pass-through TWv: fills the DMA-bound startup window
        if tails:
            twv_seg(tails[0][0], tails[0][1], 0, "t0")
        if NT > NTA:
            nc.sync.dma_start(dm_sb[:, NTA:NT, :],
                              dm2.rearrange("t p w -> p t w")[:, NTA:NT, :])
        for j in range(2, nch[1] if NPAIR > 1 else 0):
            A(1, j)
        if len(kpref) > 1:
            nc.sync.dma_start(xk_sb[:, kpref[0]:kpref[1], :],
                              xkc_r[:, kpref[0]:kpref[1], :])
        if NPAIR > 1:
            AX(1)
        nc.sync.dma_start(wv_sb[:, :, 512:1024], wvp[:, :, 512:1024])
        for i, k in enumerate(kpref[2:], 2):
            nc.sync.dma_start(xk_sb[:, kpref[i - 1]:k, :],
                              xkc_r[:, kpref[i - 1]:k, :])
        for pr in range(2, NPAIR):
            for j in range(nch[pr]):
                if (pr, j) not in hooked:
                    A(pr, j)
            AX(pr)
        if nqb:
            nc.scalar.dma_start(den[:], den_sb[:])
        if tails:
            twv_seg(tails[0][0], tails[0][1], 1, "t0")
        for c0, c1 in tails[1:]:
            twv_seg(c0, c1, 0, f"t{c0}")
            twv_seg(c0, c1, 1, f"t{c0}")
        # attention-column TWv segments
        for pr in range(0, NPAIR):
            wq_ = pblocks(pr) * P
            c0, c1 = pr * 256, pr * 256 + wq_
            last = pr == NPAIR - 1
            twv_seg(c0, c1, 0, f"a{pr}")
            twv_seg(c0, c1, 1, f"a{pr}", fine=last)

    nc.compile()
    return nc, dmidx


def _get_nc(params=None):
    with _BUILD_LOCK:
        if params is None:
            if "nc" in _CACHE:
                return _CACHE["nc"]
            params = _CACHE.get("params")
            if params is None:
                raise RuntimeError("call kernel() first to JIT the program")
        if _CACHE.get("params") != params or "nc" not in _CACHE:
            _CACHE["params"] = params
            _CACHE["nc"], _CACHE["dm_order"] = _build(params)
        return _CACHE["nc"]


def kernel(x, cross, Wq, Wk, Wv, mask):
    from concourse import bass_utils

    bf = ml_dtypes.bfloat16
    x = np.asarray(x, dtype=np.float32)
    cross = np.asarray(cross, dtype=np.float32)
    scale = 1.0 / math.sqrt(DA)
    mf = np.asarray(mask).astype(np.float32)

    params = _derive_params(mf)
    nc = _get_nc(params)
    nkb, nqb, KB, CHUNKS, MASKED = params
    NK = max(nkb, 1) * P
    NPAIR = (nqb + 1) // 2
    NT = max(sum(2 * sum(mj) for mj in MASKED), 1)

    def pack_w(wT, m_cols):
        return np.ascontiguousarray(
            wT.reshape(KC, P, m_cols).transpose(1, 0, 2)).astype(bf)

    wqp_h = pack_w((np.asarray(Wq, np.float32) * scale).T, DA)
    wkp_h = pack_w(np.asarray(Wk, np.float32).T, DA)
    wvp_h = pack_w(np.asarray(Wv, np.float32).T, D)

    in_maps = []
    rows_per_core = []
    for core in range(NCORES):
        b, p = divmod(core, 2)
        mb = mf[b] > 0
        ck = np.cumsum(mb)
        active = np.nonzero(mb)[0]
        nk = len(active)
        uq, mq = _strip_queries(mb, p)
        rows = np.concatenate([uq, mq])
        nu = len(uq)
        rows_per_core.append((b, rows, nu))
        xkc_h = np.zeros((NK, D), np.float32)
        xkc_h[:nk] = x[b][active]
        cTc_h = np.zeros((D, NK), np.float32)
        cTc_h[:, :nk] = cross[b].T[:, active]
        # additive causal/pad masks in compacted key coords, per masked chunk
        dm_h = np.full((NT, P, 512), -BIG, np.float32)
        ck_rows = np.zeros(max(nqb, 1) * P, np.int64)
        nqr = min(len(rows), nqb * P)
        ck_rows[:nqr] = ck[rows[:nqr]]
        for (pr, j, blk), ti in _CACHE["dm_order"].items():
            w = CHUNKS[pr][j]
            base = sum(CHUNKS[pr][:j])
            ckb = ck_rows[(pr * 2 + blk) * P:(pr * 2 + blk + 1) * P]
            kidx = base + np.arange(w)
            dm_h[ti, :, :w] = np.where(
                kidx[None, :] < ckb[:, None], 0.0, -BIG)
        # predication mask: 1 = attention column is a real unmasked query
        pm_h = np.zeros((max(NPAIR, 1), 256), np.uint8)
        for pr in range(NPAIR):
            c0 = pr * 256
            n1 = min(max(nu - c0, 0), 256)
            pm_h[pr, :n1] = 1
        pm_h = np.broadcast_to(pm_h[None, :, :], (P, max(NPAIR, 1), 256))
        in_maps.append({
            "xkc": xkc_h.astype(bf),
            "cTc": cTc_h.astype(bf),
            "xqT": np.ascontiguousarray(x[b][rows].T).astype(bf),
            "wqp": wqp_h,
            "wkp": wkp_h,
            "wvp": wvp_h,
            "dm2": dm_h.astype(bf),
            "pmask": np.ascontiguousarray(pm_h),
        })

    _CACHE["in_maps"] = in_maps
    res = bass_utils.run_bass_kernel_spmd(
        nc, in_maps, core_ids=list(range(NCORES)))

    out = np.empty((B, S, D), np.float32)
    for core in range(NCORES):
        b, rows, nu = rows_per_core[core]
        r = res.results[core]
        o = r["outT"].T.astype(np.float32)  # [1024 q, 1024 dm]
        denf = np.ones(NQ, np.float32)
        if nqb:
            dev_den = r["den"].T.reshape(-1)  # [nqb*128] col-ordered
            denf[:nu] = dev_den[:nu]
        out[b, rows] = o / denf[:, None]
    return out


# revision 34
# speedup vs baseline: 1.0059x; 1.0059x over previous
"""Trainium2 Bass kernel for nn_Attention_42288247996512 (sparse causal cross-attention).

reference:
  q = x @ Wq.T; k = cross @ Wk.T; v = x @ Wv.T
  logits = q @ k.T  (causal mask; padding mask m_q*m_k + eye > 0)
  out = softmax(logits / sqrt(128)) @ v

Sharding: 8 cores = 4 batches x 2 query-strips (SPMD), 1024 query rows per
core.  The strip's column order is host-chosen: [unmasked queries | masked
queries], with unmasked queries split between the batch's two strips by
even/odd compacted index so both strips have near-identical causal
structure.

Structural optimizations vs a vanilla flash-style kernel:

1) Reassociation:  attn @ (x @ Wv.T) == (attn @ x) @ Wv.T.  Each core owns
   1024 query rows but would need all 2048 key rows of v, so projecting
   t = attn@x instead of v halves that matmul.

2) Key compaction: ~half the keys are padding-masked (exp == 0 columns).
   Keys are host-compacted to the active ones, shrinking
   kT/logits/exp/transpose/AX nearly 2x.

3) Query compaction: masked queries have softmax == delta(diagonal), i.e.
   out[q] = v[q].  Attention (qT/logits/exp/AX) runs only on the ~unmasked
   column prefix; x.T is DMA'd into the t buffer so the masked columns pass
   through t @ Wv.T and produce v[q] directly.  Attention results are merged
   over the pass-through with predicated copies (mask = column-is-unmasked).
   The pure-pass-through tail columns' t @ Wv.T segment has no attention
   dependency, providing early PE work that overlaps the input DMA stream.

The kernel program is JIT-specialized on the mask's *structure* (block
counts / chunk widths / which chunks need additive masks, maxed over the 8
cores so one SPMD program serves all).  Structure parameters are recomputed
from the input mask on every call (and cached), so the kernel stays correct
for any input.  All streamed operands are bf16; PSUM accumulation is f32.
Host does layout packs/bf16 casts, gathers, additive-mask building, and the
final denominator divide + scatter (as in the baseline kernel).
"""
import math
import threading

import ml_dtypes
import numpy as np

B, S, D, DA = 4, 2048, 1024, 128
P = 128
NCORES = 8
BIG = 32768.0  # power of two: exactly representable in bf16
NQ = 1024      # query rows per core strip
KC = D // P    # 8 contraction chunks of 128

_BUILD_LOCK = threading.Lock()
_CACHE: dict = {}


def _strip_queries(mb, p):
    """Column order for strip p of a batch: unmasked (even/odd compacted
    interleave) first, then masked, filled so each strip has exactly NQ
    rows and the two strips partition the batch's 2048 rows."""
    uq = np.nonzero(mb)[0]
    mq = np.nonzero(~mb)[0]
    u = uq[p::2]
    w0 = NQ - len(uq[0::2])
    mtake = mq[:w0] if p == 0 else mq[w0:]
    return u, mtake


def _derive_params(mask_f):
    """SPMD kernel structure (maxed over all 8 cores) from the mask.

    Returns (nkb, nqb, kb, chunks, masked):
      nkb:   compacted key blocks (128 each)
      nqb:   attention query blocks per strip (128 each)
      kb:    per-pair key-block count (AX contraction length)
      chunks: per-pair tuple of logits chunk widths (<=512, multiples of 128)
      masked: per-pair tuple of bools - does chunk j need an additive mask
    """
    nkb = 0
    nqb = 0
    stats = []  # per (b, p): (ck, uq, nk)
    for b in range(B):
        m = mask_f[b] > 0
        ck = np.cumsum(m)
        nk = int(ck[-1])
        nkb = max(nkb, (nk + P - 1) // P)
        for p in range(2):
            uq, _ = _strip_queries(m, p)
            nqb = max(nqb, (len(uq) + P - 1) // P)
            stats.append((ck, uq, nk))
    npair = (nqb + 1) // 2
    kb = [0] * npair
    for ck, uq, nk in stats:
        for pr in range(npair):
            qs = uq[pr * 256:min((pr + 1) * 256, len(uq))]
            if len(qs) == 0:
                continue
            bmax = int(ck[qs].max())
            kb[pr] = max(kb[pr], (bmax + P - 1) // P)
    chunks = []
    for pr in range(npair):
        w = kb[pr] * P
        ch = []
        while w > 0:
            ch.append(min(512, w))
            w -= min(512, w)
        chunks.append(tuple(ch))
    masked = [[False] * len(chunks[pr]) for pr in range(npair)]
    for ck, uq, nk in stats:
        for pr in range(npair):
            qs = uq[pr * 256:min((pr + 1) * 256, len(uq))]
            bmin = int(ck[qs].min()) if len(qs) else 0
            base = 0
            for j, w in enumerate(chunks[pr]):
                if base + w > bmin or base + w > nk:
                    masked[pr][j] = True
                base += w
    return (nkb, nqb, tuple(kb), tuple(chunks),
            tuple(tuple(mj) for mj in masked))


def _build(params):
    from contextlib import ExitStack

    import concourse.bass as bass
    import concourse.mybir as mybir
    import concourse.tile as tile
    from concourse import bacc
    from concourse.masks import make_identity

    nkb, nqb, KB, CHUNKS, MASKED = params
    NK = max(nkb, 1) * P             # padded compacted key width
    NPAIR = (nqb + 1) // 2           # attention column pairs (<=256 each)
    QCOL = nqb * P                   # attention columns
    nt = sum(2 * sum(mj) for mj in MASKED)
    NT = max(nt, 1)

    def pblocks(pr):
        return 1 if (pr == NPAIR - 1 and nqb % 2 == 1) else 2

    dt = mybir.dt
    f32 = dt.float32
    bf16 = dt.bfloat16
    u8 = dt.uint8
    AF = mybir.ActivationFunctionType
    ALU = mybir.AluOpType

    nc = bacc.Bacc("TRN2", target_bir_lowering=False, debug=False)

    xkc = nc.dram_tensor("xkc", [NK, D], bf16, kind="ExternalInput").ap()
    cTc = nc.dram_tensor("cTc", [D, NK], bf16, kind="ExternalInput").ap()
    xqT = nc.dram_tensor("xqT", [D, NQ], bf16, kind="ExternalInput").ap()
    wqp = nc.dram_tensor("wqp", [P, KC, DA], bf16, kind="ExternalInput").ap()
    wkp = nc.dram_tensor("wkp", [P, KC, DA], bf16, kind="ExternalInput").ap()
    wvp = nc.dram_tensor("wvp", [P, KC, D], bf16, kind="ExternalInput").ap()
    dm2 = nc.dram_tensor("dm2", [NT, P, 512], bf16, kind="ExternalInput").ap()
    pmask = nc.dram_tensor("pmask", [P, max(NPAIR, 1), 256], u8,
                           kind="ExternalInput").ap()

    outT = nc.dram_tensor("outT", [D, NQ], bf16,
                          kind="ExternalOutput").ap()
    den = nc.dram_tensor("den", [P, max(nqb, 1)], f32,
                         kind="ExternalOutput").ap()

    xkc_r = xkc.rearrange("(kb p) d -> p kb d", p=P)
    cTc_r = cTc.rearrange("(kc p) s -> p kc s", p=P)
    xqT_r = xqT.rearrange("(kc p) q -> p kc q", p=P)
    outT_r = outT.rearrange("(dmc p) q -> p dmc q", p=P)

    dmidx = {}

    def dm_tile(pr, j, blk):
        key = (pr, j, blk)
        if key not in dmidx:
            dmidx[key] = len(dmidx)
        return dmidx[key]

    kt_chunks = []
    w = NK
    while w > 0:
        kt_chunks.append(min(512, w))
        w -= min(512, w)

    with tile.TileContext(nc) as tc, ExitStack() as ctx:
        const = ctx.enter_context(tc.tile_pool(name="const", bufs=1))
        persist = ctx.enter_context(tc.tile_pool(name="persist", bufs=1))
        stream = ctx.enter_context(tc.tile_pool(name="stream", bufs=2))
        apool = ctx.enter_context(tc.tile_pool(name="apool", bufs=4))
        epool = ctx.enter_context(tc.tile_pool(name="epool", bufs=24))

        ident_f32 = const.tile([P, P], f32, name="ident_f32")
        make_identity(nc, ident_f32)
        ident = const.tile([P, P], bf16, name="ident")
        nc.vector.tensor_copy(ident[:], ident_f32[:])

        wq_sb = const.tile([P, KC, DA], bf16, name="wq_sb")
        wk_sb = const.tile([P, KC, DA], bf16, name="wk_sb")
        wv_sb = const.tile([P, KC, D], bf16, name="wv_sb")
        dm_sb = const.tile([P, NT, 512], bf16, name="dm_sb")
        pm_sb = const.tile([P, max(NPAIR, 1), 256], u8, name="pm_sb")

        kT_sb = persist.tile([P, NK], bf16, name="kT_sb")
        qT_sb = persist.tile([P, max(QCOL, P)], bf16, name="qT_sb")
        xk_sb = persist.tile([P, max(nkb, 1), D], bf16, name="xk_sb")
        tT_sb = persist.tile([P, KC, NQ], bf16, name="tT_sb")
        den_sb = persist.tile([P, max(nqb, 1)], f32, name="den_sb")

        eTs_all = {pr: [] for pr in range(NPAIR)}
        daccs_all = {pr: [[], []] for pr in range(NPAIR)}

        psl_pool = ctx.enter_context(
            tc.tile_pool(name="psl", bufs=2, space="PSUM"))
        psT_pool = ctx.enter_context(
            tc.tile_pool(name="psT", bufs=2, space="PSUM"))
        psax_pool = ctx.enter_context(
            tc.tile_pool(name="psax", bufs=2, space="PSUM"))
        psw_pool = ctx.enter_context(
            tc.tile_pool(name="psw", bufs=2, space="PSUM"))

        # PE warmup: dependency-free matmuls keep the PE busy (and its
        # p-state at full clock) while the first input DMAs stream in.
        _wu = [0]

        def warm(n):
            for _ in range(n):
                pswu = psw_pool.tile([P, P], f32, tag="psw",
                                     name=f"pswu{_wu[0]}",
                                     padded_shape=[P, 512])
                _wu[0] += 1
                nc.tensor.matmul(pswu[:], lhsT=ident[:], rhs=ident[:],
                                 start=True, stop=True)

        warm(48)

        # ---- projections (DMA emission order == SP FIFO delivery order) ----
        def kT_chunk(j):
            if nkb == 0:
                return
            w = kt_chunks[j]
            base = sum(kt_chunks[:j])
            ctj = stream.tile([P, KC, 512], bf16, tag="ct", name=f"ct{j}",
                              bufs=2)
            nc.sync.dma_start(ctj[:, :, :w], cTc_r[:, :, base:base + w])
            ps_k = psax_pool.tile([P, 512], f32, tag="psax", name=f"ps_k{j}")
            for kc in range(KC):
                nc.tensor.matmul(
                    ps_k[:, :w],
                    lhsT=wk_sb[:, kc, :],
                    rhs=ctj[:, kc, :w],
                    start=(kc == 0), stop=(kc == KC - 1),
                )
            nc.any.tensor_copy(kT_sb[:, base:base + w], ps_k[:, :w])

        nc.sync.dma_start(wq_sb[:], wqp)
        # qT over the attention columns only; rhs is the tT fill itself
        qch = []
        w = QCOL
        while w > 0:
            qch.append(min(512, w))
            w -= min(512, w)
        ps_q = [psl_pool.tile([P, 512], f32, tag="psl", name=f"ps_q{n}")
                for n in range(len(qch))]

        def qT_fill(c0, c1):
            nc.sync.dma_start(tT_sb[:, :, c0:c1], xqT_r[:, :, c0:c1])

        def qT_mm(n):
            base = sum(qch[:n])
            for kc in range(KC):
                nc.tensor.matmul(
                    ps_q[n][:, :qch[n]],
                    lhsT=wq_sb[:, kc, :],
                    rhs=tT_sb[:, kc, base:base + qch[n]],
                    start=(kc == 0), stop=(kc == KC - 1),
                )

        if qch:
            qT_fill(0, qch[0])
            qT_mm(0)
        nc.sync.dma_start(wk_sb[:], wkp)
        kT_chunk(0)
        for n in range(1, len(qch)):
            base = sum(qch[:n])
            qT_fill(base, base + qch[n])
            qT_mm(n)
        for n in range(len(qch)):
            base = sum(qch[:n])
            nc.any.tensor_copy(qT_sb[:, base:base + qch[n]],
                               ps_q[n][:, :qch[n]])
        NTA = min(6, NT)
        nc.sync.dma_start(dm_sb[:, 0:NTA, :],
                          dm2.rearrange("t p w -> p t w")[:, 0:NTA, :])
        nc.sync.dma_start(pm_sb[:], pmask)

        # ---- attention stages ----
        def stage_a_chunk(pr, j, mid_hook=None):
            w = CHUNKS[pr][j]
            base = sum(CHUNKS[pr][:j])
            nks = (w + P - 1) // P
            nbl = pblocks(pr)
            psTs = [psT_pool.tile([P, nbl * P], bf16, tag="psT",
                                  name=f"psT{pr}_{j}_{ks}",
                                  padded_shape=[P, 512])
                    for ks in range(nks)]
            es = []
            for blk in range(nbl):
                slot = pr * 2 + blk
                psl = psl_pool.tile([P, 512], f32, tag="psl",
                                    name=f"psl{slot}_{j}")
                nc.tensor.matmul(
                    psl[:, :w],
                    lhsT=qT_sb[:, slot * P:(slot + 1) * P],
                    rhs=kT_sb[:, base:base + w],
                    start=True, stop=True,
                )
                dac = apool.tile([P, 1], f32, tag="dac",
                                 name=f"dac{slot}_{j}", bufs=12)
                e = apool.tile([P, 512], bf16, tag="e", name=f"e{slot}_{j}")
                if MASKED[pr][j]:
                    sbl = apool.tile([P, 512], f32, tag="sbl",
                                     name=f"sbl{slot}_{j}")
                    nc.vector.tensor_tensor(
                        out=sbl[:, :w], in0=psl[:, :w],
                        in1=dm_sb[:, dm_tile(pr, j, blk), :w], op=ALU.add)
                    src = sbl
                else:
                    src = psl
                if w > 256:
                    dac2 = apool.tile([P, 1], f32, tag="dac",
                                      name=f"dac2_{slot}_{j}", bufs=12)
                    nc.scalar.activation(
                        e[:, :256], src[:, :256], AF.Exp,
                        bias=0.0, scale=1.0, accum_out=dac[:])
                    nc.scalar.activation(
                        e[:, 256:w], src[:, 256:w], AF.Exp,
                        bias=0.0, scale=1.0, accum_out=dac2[:])
                    daccs_all[pr][blk].append(dac2)
                else:
                    nc.scalar.activation(
                        e[:, :w], src[:, :w], AF.Exp,
                        bias=0.0, scale=1.0, accum_out=dac[:])
                daccs_all[pr][blk].append(dac)
                es.append(e)
            if mid_hook is not None:
                mid_hook()
            for blk in range(nbl):
                for ks in range(nks):
                    nc.tensor.transpose(
                        psTs[ks][:, blk * P:(blk + 1) * P],
                        es[blk][:, ks * P:(ks + 1) * P],
                        ident[:],
                    )
            for ks in range(nks):
                eT = epool.tile([P, nbl * P], bf16, tag="eT",
                                name=f"eT{pr}_{j}_{ks}")
                nc.any.tensor_copy(eT[:], psTs[ks][:])
                eTs_all[pr].append(eT)

        def stage_den(pr):
            for blk in range(pblocks(pr)):
                slot = pr * 2 + blk
                dl = daccs_all[pr][blk]
                dst = den_sb[:, slot:slot + 1]
                if len(dl) == 1:
                    nc.any.tensor_copy(dst, dl[0][:])
                else:
                    nc.vector.tensor_tensor(
                        out=dst, in0=dl[0][:], in1=dl[1][:], op=ALU.add)
                    for d in dl[2:]:
                        nc.vector.tensor_tensor(
                            out=dst, in0=dst, in1=d[:], op=ALU.add)

        def stage_ax(pr, mid_hook=None):
            stage_den(pr)
            eTs = eTs_all[pr]
            wq_ = pblocks(pr) * P
            for xdc in range(KC):
                psax = psax_pool.tile([P, wq_], f32, tag="psax",
                                      name=f"psax{pr}_{xdc}",
                                      padded_shape=[P, 512])
                for kb in range(KB[pr]):
                    nc.tensor.matmul(
                        psax[:],
                        lhsT=xk_sb[:, kb, xdc * P:(xdc + 1) * P],
                        rhs=eTs[kb][:],
                        start=(kb == 0), stop=(kb == KB[pr] - 1),
                    )
                nc.vector.copy_predicated(
                    tT_sb[:, xdc, pr * 256:pr * 256 + wq_],
                    pm_sb[:, pr, :wq_], psax[:])
                if mid_hook is not None and xdc == 3:
                    mid_hook()

        _osbs = {}

        def twv_seg(c0, c1, dm_half, osb_key, fine=False, den_dma=False):
            # out.T[dm, c0:c1] = Wv.T.T @ t.T columns [c0, c1)
            wseg = c1 - c0
            osb = _osbs.setdefault(
                osb_key, apool.tile([P, KC, 512], bf16, tag="osb",
                                    name=f"osb{osb_key}", bufs=2))
            del den_dma
            for dmc in range(4 * dm_half, 4 * dm_half + 4):
                psw = psw_pool.tile([P, 512], f32, tag="psw",
                                    name=f"psw{osb_key}_{dmc}")
                for xdc in range(KC):
                    nc.tensor.matmul(
                        psw[:, :wseg],
                        lhsT=wv_sb[:, xdc, dmc * P:(dmc + 1) * P],
                        rhs=tT_sb[:, xdc, c0:c1],
                        start=(xdc == 0), stop=(xdc == KC - 1),
                    )
                if dmc % 2 == 0:
                    nc.vector.tensor_copy(osb[:, dmc, :wseg],
                                          psw[:, :wseg])
                else:
                    nc.scalar.copy(osb[:, dmc, :wseg], psw[:, :wseg])
                if fine:
                    eng = nc.scalar if dmc % 2 == 0 else nc.sync
                    eng.dma_start(
                        outT_r[:, dmc:dmc + 1, c0:c1],
                        osb[:, dmc:dmc + 1, :wseg])
            if not fine:
                nc.scalar.dma_start(
                    outT_r[:, 4 * dm_half:4 * dm_half + 4, c0:c1],
                    osb[:, 4 * dm_half:4 * dm_half + 4, :wseg])

        def A(pr, j, mid_hook=None):
            if pr < NPAIR and j < len(CHUNKS[pr]) and CHUNKS[pr][j] > 0:
                stage_a_chunk(pr, j, mid_hook=mid_hook)
            elif mid_hook is not None:
                mid_hook()

        def AX(pr, mid_hook=None):
            if pr < NPAIR and KB[pr] > 0:
                stage_ax(pr, mid_hook=mid_hook)
            elif mid_hook is not None:
                mid_hook()

        # ---- fused schedule (PE emission order tuned to DMA arrivals) ----
        tails = []
        c = QCOL
        while c < NQ:
            c2 = min(c + 512, NQ)
            tails.append((c, c2))
            c = c2

        nch = [len(CHUNKS[pr]) for pr in range(NPAIR)]
        kpref = sorted(set(
            [min(KB[pr], nkb) for pr in range(NPAIR)] + [nkb]))
        kpref = [k for k in kpref if k > 0]

        # pull an unmasked chunk of a later pair into AX(0)'s copy stalls
        hooked = []

        def ax0_hook():
            for pr in range(2, NPAIR):
                if nch[pr] > 0 and not MASKED[pr][0]:
                    hooked.append((pr, 0))
                    A(pr, 0)
                    return

        A(0, 0, mid_hook=lambda: kT_chunk(1) if len(kt_chunks) > 1 else None)
        if NPAIR > 1:
            A(1, 0, mid_hook=lambda: [kT_chunk(j) for j in
                                      range(2, len(kt_chunks))])
            A(1, 1)
        if kpref:
            nc.sync.dma_start(xk_sb[:, 0:kpref[0], :],
                              xkc_r[:, 0:kpref[0], :])
        # pure pass-through tail fill (no attention dependency)
        if QCOL < NQ:
            nc.sync.dma_start(tT_sb[:, :, QCOL:NQ], xqT_r[:, :, QCOL:NQ])
        AX(0, mid_hook=ax0_hook)
        nc.sync.dma_start(wv_sb[:, :, 0:512], wvp[:, :, 0:512])
        # early pure-pass-through TWv: fills the DMA-bound startup window
        if tails:
            twv_seg(tails[0][0], tails[0][1], 0, "t0")
        if NT > NTA:
            nc.sync.dma_start(dm_sb[:, NTA:NT, :],
                              dm2.rearrange("t p w -> p t w")[:, NTA:NT, :])
        for j in range(2, nch[1] if NPAIR > 1 else 0):
            A(1, j)
        if len(kpref) > 1:
            nc.sync.dma_start(xk_sb[:, kpref[0]:kpref[1], :],
                              xkc_r[:, kpref[0]:kpref[1], :])
        if NPAIR > 1:
            AX(1)
        nc.sync.dma_start(wv_sb[:, :, 512:1024], wvp[:, :, 512:1024])
        for i, k in enumerate(kpref[2:], 2):
            nc.sync.dma_start(xk_sb[:, kpref[i - 1]:k, :],
                              xkc_r[:, kpref[i - 1]:k, :])
        for pr in range(2, NPAIR):
            for j in range(nch[pr]):
                if (pr, j) not in hooked:
                    A(pr, j)
            AX(pr)
        if nqb:
            nc.scalar.dma_start(den[:], den_sb[:])
        if tails:
            twv_seg(tails[0][0], tails[0][1], 1, "t0")
        for c0, c1 in tails[1:]:
            twv_seg(c0, c1, 0, f"t{c0}")
            twv_seg(c0, c1, 1, f"t{c0}")
        # attention-column TWv segments
        for pr in range(0, NPAIR):
            wq_ = pblocks(pr) * P
            c0, c1 = pr * 256, pr * 256 + wq_
            last = pr == NPAIR - 1
            twv_seg(c0, c1, 0, f"a{pr}")
            twv_seg(c0, c1, 1, f"a{pr}", fine=last)

    nc.compile()
    return nc, dmidx


def _get_nc(params=None):
    with _BUILD_LOCK:
        if params is None:
            if "nc" in _CACHE:
                return _CACHE["nc"]
            params = _CACHE.get("params")
            if params is None:
                raise RuntimeError("call kernel() first to JIT the program")
        if _CACHE.get("params") != params or "nc" not in _CACHE:
            _CACHE["params"] = params
            _CACHE["nc"], _CACHE["dm_order"] = _build(params)
        return _CACHE["nc"]


def kernel(x, cross, Wq, Wk, Wv, mask):
    from concourse import bass_utils

    bf = ml_dtypes.bfloat16
    x = np.asarray(x, dtype=np.float32)
    cross = np.asarray(cross, dtype=np.float32)
    scale = 1.0 / math.sqrt(DA)
    mf = np.asarray(mask).astype(np.float32)

    params = _derive_params(mf)
    nc = _get_nc(params)
    nkb, nqb, KB, CHUNKS, MASKED = params
    NK = max(nkb, 1) * P
    NPAIR = (nqb + 1) // 2
    NT = max(sum(2 * sum(mj) for mj in MASKED), 1)

    def pack_w(wT, m_cols):
        return np.ascontiguousarray(
            wT.reshape(KC, P, m_cols).transpose(1, 0, 2)).astype(bf)

    wqp_h = pack_w((np.asarray(Wq, np.float32) * scale).T, DA)
    wkp_h = pack_w(np.asarray(Wk, np.float32).T, DA)
    wvp_h = pack_w(np.asarray(Wv, np.float32).T, D)

    in_maps = []
    rows_per_core = []
    for core in range(NCORES):
        b, p = divmod(core, 2)
        mb = mf[b] > 0
        ck = np.cumsum(mb)
        active = np.nonzero(mb)[0]
        nk = len(active)
        uq, mq = _strip_queries(mb, p)
        rows = np.concatenate([uq, mq])
        nu = len(uq)
        rows_per_core.append((b, rows, nu))
        xkc_h = np.zeros((NK, D), np.float32)
        xkc_h[:nk] = x[b][active]
        cTc_h = np.zeros((D, NK), np.float32)
        cTc_h[:, :nk] = cross[b].T[:, active]
        # additive causal/pad masks in compacted key coords, per masked chunk
        dm_h = np.full((NT, P, 512), -BIG, np.float32)
        ck_rows = np.zeros(max(nqb, 1) * P, np.int64)
        nqr = min(len(rows), nqb * P)
        ck_rows[:nqr] = ck[rows[:nqr]]
        for (pr, j, blk), ti in _CACHE["dm_order"].items():
            w = CHUNKS[pr][j]
            base = sum(CHUNKS[pr][:j])
            ckb = ck_rows[(pr * 2 + blk) * P:(pr * 2 + blk + 1) * P]
            kidx = base + np.arange(w)
            dm_h[ti, :, :w] = np.where(
                kidx[None, :] < ckb[:, None], 0.0, -BIG)
        # predication mask: 1 = attention column is a real unmasked query
        pm_h = np.zeros((max(NPAIR, 1), 256), np.uint8)
        for pr in range(NPAIR):
            c0 = pr * 256
            n1 = min(max(nu - c0, 0), 256)
            pm_h[pr, :n1] = 1
        pm_h = np.broadcast_to(pm_h[None, :, :], (P, max(NPAIR, 1), 256))
        in_maps.append({
            "xkc": xkc_h.astype(bf),
            "cTc": cTc_h.astype(bf),
            "xqT": np.ascontiguousarray(x[b][rows].T).astype(bf),
            "wqp": wqp_h,
            "wkp": wkp_h,
            "wvp": wvp_h,
            "dm2": dm_h.astype(bf),
            "pmask": np.ascontiguousarray(pm_h),
        })

    _CACHE["in_maps"] = in_maps
    res = bass_utils.run_bass_kernel_spmd(
        nc, in_maps, core_ids=list(range(NCORES)))

    out = np.empty((B, S, D), np.float32)
    for core in range(NCORES):
        b, rows, nu = rows_per_core[core]
        r = res.results[core]
        o = r["outT"].T.astype(np.float32)  # [1024 q, 1024 dm]
        denf = np.ones(NQ, np.float32)
        if nqb:
            dev_den = r["den"].T.reshape(-1)  # [nqb*128] col-ordered
            denf[:nu] = dev_den[:nu]
        out[b, rows] = o / denf[:, None]
    return out
